# revision 1
# baseline (speedup 1.0000x reference)
"""Trainium2 Bass kernel for nn_AdditiveCouplingLayer (additive coupling + 5-block
BatchNorm MLP), data-parallel over 8 NeuronCores.

Strategy:
  - Shard batch (16384) across 8 cores (2048 rows each); weights replicated.
  - Keep activations TRANSPOSED on chip: h^T is [hidden, batch] so hidden units
    map to SBUF partitions. BatchNorm stats are then free-dim reductions
    (bn_stats/bn_aggr), and the per-layer matmul h' = W^T @ h^T uses the weight
    matrices exactly as stored (lhsT = W[k,m] stationary, rhs = h^T moving).
  - Matmuls run in float32r (TF32-like: bf16 PE throughput, ~1.5e-4 rel err).
    DVE instructions produce the f32r-rounded operand tiles; weights are
    DMA'd into f32r-typed tiles straight from f32r-typed DRAM inputs.
  - BatchNorm batch stats cross the 8 cores via a small AllGather + local DVE
    reduce (lower latency than AllReduce); a dummy warm-up collective at kernel
    start absorbs the ~65us first-collective ncfw cold cost.
  - Input extraction (x[:, 0::2] -> x1^T) and the output re-transpose are PE
    transposes interleaved chunk-wise with the neighboring matmul phases so the
    PE stays busy and warm.
"""

import sys

sys.path.insert(0, "/opt/trn_rl_repo")

import numpy as np

BN_EPS = 1e-5

# Full-problem constants
B_FULL, D_FULL, H_FULL, NL_FULL, NCORES = 16384, 784, 1024, 5, 8


def build_kernel(B=B_FULL, D=D_FULL, H=H_FULL, NL=NL_FULL, n_cores=NCORES):
    import concourse.bacc as bacc
    import concourse.mybir as mybir
    from concourse import tile, masks

    f32 = mybir.dt.float32
    f32r = mybir.dt.float32r
    AF = mybir.ActivationFunctionType
    ALU = mybir.AluOpType
    AX = mybir.AxisListType

    L = D // 2                     # latent width (coupling half)
    C = B // n_cores               # batch rows per core
    LT = (L + 127) // 128          # latent k-tiles
    LREM = L - (LT - 1) * 128      # width of last latent tile
    MT = H // 128                  # hidden m/k tiles
    NCHW = min(512, C)             # moving free-dim chunk
    NCH = C // NCHW                # chunks per row-block
    BT = C // 128                  # batch tiles of 128 rows
    BPC = NCHW // 128              # batch tiles per chunk

    nc = bacc.Bacc("TRN2", target_bir_lowering=False, debug=False,
                   num_devices=n_cores)

    x_d = nc.dram_tensor("x", [C, D], f32, kind="ExternalInput")
    win_d = nc.dram_tensor("win", [LT * 128, H], f32r, kind="ExternalInput")
    wh_d = nc.dram_tensor("wh", [NL, H, H], f32r, kind="ExternalInput")
    wout_d = nc.dram_tensor("wout", [H, L], f32r, kind="ExternalInput")
    bin_d = nc.dram_tensor("bin", [H], f32, kind="ExternalInput")
    bh_d = nc.dram_tensor("bh", [NL, H], f32, kind="ExternalInput")
    gamma_d = nc.dram_tensor("gamma", [NL, H], f32, kind="ExternalInput")
    beta_d = nc.dram_tensor("beta", [NL, H], f32, kind="ExternalInput")
    bout_d = nc.dram_tensor("bout", [LT * 128], f32, kind="ExternalInput")
    out_d = nc.dram_tensor("out", [C, D], f32, kind="ExternalOutput")

    rg = [list(range(n_cores))]

    with tile.TileContext(nc) as tc:
        with (
            tc.tile_pool(name="w", bufs=1) as wp,
            tc.tile_pool(name="h", bufs=1) as hp,
            tc.tile_pool(name="xio", bufs=2) as xp,
            tc.tile_pool(name="small", bufs=2) as sp,
            tc.tile_pool(name="psum", bufs=4, space="PSUM") as pp,
            tc.tile_pool(name="dram", bufs=2, space="DRAM") as dp,
            tc.tile_pool(name="const", bufs=1) as cp,
        ):
            ident = cp.tile([128, 128], f32)
            masks.make_identity(nc, ident[:])
            zrow = cp.tile([128, NCHW], f32)
            nc.vector.memset(zrow[:], 0.0)
            # ~3.4us of dummy matmuls: fills the PE activity window during the
            # DMA lead-in so real work starts at full clock (HAM un-throttle)
            zrow_r = cp.tile([128, NCHW], f32r)
            nc.vector.tensor_copy(zrow_r[:], zrow[:])
            for wu in range(14):
                psw = pp.tile([128, NCHW], f32, tag="mm", name=f"warmmm{wu}")
                nc.tensor.matmul(psw[:], zrow_r[:, 0:128], zrow_r[:])

            # Warm-up collective: absorbs the first-collective ncfw cold start
            # on the CC cores, fully overlapped with the input stage.
            for wu in range(4):
                warm_in = dp.tile([128, 16], f32, tag="warmin",
                                  name=f"warmin{wu}")
                warm_out = dp.tile([n_cores * 128, 16], f32, tag="warmout",
                                   name=f"warmout{wu}", addr_space="Shared")
                nc.gpsimd.dma_start(warm_in[:], zrow[:, 0:16])
                nc.gpsimd.collective_compute(
                    "AllGather", ALU.bypass, replica_groups=rg,
                    ins=[warm_in.opt()], outs=[warm_out.opt()])

            # ---- Stage A+B interleaved per chunk:
            #   x1^T = transpose(x[:, 0::2]);  h0^T = Win^T @ x1^T + bin ----
            wt = [wp.tile([128, H], f32r, tag=f"w{k}", name=f"wtin_{k}")
                  for k in range(LT)]
            binT = sp.tile([128, MT], f32, tag="biasT")

            x1 = [hp.tile([128, C], f32r, tag=f"ha{j}", name=f"x1_{j}")
                  for j in range(LT)]
            cur = [hp.tile([128, C], f32r, tag=f"hb{m}", name=f"h0_{m}")
                   for m in range(MT)]
            for n in range(NCH):
                ncs = slice(n * NCHW, (n + 1) * NCHW)
                if LREM < 128:
                    # zero-fill padded k rows via a rounding DVE producer
                    # (full-partition write; valid rows overwritten below)
                    nc.vector.tensor_copy(x1[LT - 1][:, ncs], zrow[:])
                for b in range(n * BPC, (n + 1) * BPC):
                    xin = xp.tile([128, D], f32, tag="xin", bufs=3)
                    if b == 0:
                        # split the first tile across 4 HWDGE queues so the
                        # input chain starts ~9us earlier
                        for q in range(4):
                            nc.sync.dma_start(
                                xin[q * 32:(q + 1) * 32, :],
                                x_d[b * 128 + q * 32:b * 128 + (q + 1) * 32, :])
                    else:
                        nc.sync.dma_start(xin[:], x_d[b * 128:(b + 1) * 128, :])
                    xe = xp.tile([128, L], f32, tag="xe")
                    nc.scalar.copy(
                        xe[:],
                        xin[:].rearrange("p (l two) -> p l two", two=2)[:, :, 0])
                    for j in range(LT):
                        wj = 128 if j < LT - 1 else LREM
                        ps = pp.tile([128, 128], f32, tag="tr")
                        nc.tensor.transpose(ps[0:wj, :],
                                            xe[:, j * 128:j * 128 + wj],
                                            ident[:])
                        nc.vector.tensor_copy(
                            x1[j][0:wj, b * 128:(b + 1) * 128], ps[0:wj, :])
                if n == 0:
                    # weight DMAs issued after chunk-0's x tiles so the input
                    # chain isn't stuck behind 2MB of weight traffic
                    for k in range(LT):
                        nc.sync.dma_start(wt[k][:],
                                          win_d[k * 128:(k + 1) * 128, :])
                    nc.sync.dma_start(
                        binT[:], bin_d[:].rearrange("(m p) -> p m", p=128))
                for m in range(MT):
                    ps = pp.tile([128, NCHW], f32, tag="mm")
                    for k in range(LT):
                        nc.tensor.matmul(ps[:], wt[k][:, m * 128:(m + 1) * 128],
                                         x1[k][:, ncs],
                                         start=(k == 0), stop=(k == LT - 1))
                    # add-bias drain alternates DVE/ACT so PSUM slots recycle
                    # at PE rate; both produce f32r-rounded h0
                    if m % 2 == 0:
                        nc.vector.tensor_scalar(
                            out=cur[m][:, ncs], in0=ps[:],
                            scalar1=binT[:, m:m + 1], scalar2=None, op0=ALU.add)
                    else:
                        nc.scalar.activation(
                            cur[m][:, ncs], ps[:], AF.Identity,
                            bias=binT[:, m:m + 1], scale=1.0)

            # ---- Hidden blocks: h = BN(relu(Wh^T @ h + bh)) ----
            for l in range(NL):
                wt = [wp.tile([128, H], f32r, tag=f"w{k}", name=f"wh_{l}_{k}")
                      for k in range(MT)]
                for k in range(MT):
                    nc.sync.dma_start(wt[k][:],
                                      wh_d[l, k * 128:(k + 1) * 128, :])
                bhT = sp.tile([128, MT], f32, tag="biasT")
                nc.sync.dma_start(bhT[:],
                                  bh_d[l, :].rearrange("(m p) -> p m", p=128))
                gT = sp.tile([128, MT], f32, tag="gT")
                nc.sync.dma_start(gT[:],
                                  gamma_d[l, :].rearrange("(m p) -> p m", p=128))
                bT = sp.tile([128, MT], f32, tag="bT")
                nc.sync.dma_start(bT[:],
                                  beta_d[l, :].rearrange("(m p) -> p m", p=128))

                outt = [hp.tile([128, C], f32, tag=f"ha{m}", name=f"hp_{l}_{m}")
                        for m in range(MT)]
                ag = sp.tile([128, 2 * MT], f32, tag="ag")
                for m in range(MT):
                    st = sp.tile([128, 6 * NCH], f32, tag="st")
                    for n in range(NCH):
                        ncs = slice(n * NCHW, (n + 1) * NCHW)
                        ps = pp.tile([128, NCHW], f32, tag="mm")
                        for k in range(MT):
                            nc.tensor.matmul(
                                ps[:], wt[k][:, m * 128:(m + 1) * 128],
                                cur[k][:, ncs],
                                start=(k == 0), stop=(k == MT - 1))
                        nc.scalar.activation(outt[m][:, ncs], ps[:], AF.Relu,
                                             bias=bhT[:, m:m + 1], scale=1.0)
                        nc.vector.bn_stats(st[:, 6 * n:6 * n + 6],
                                           outt[m][:, ncs])
                    nc.vector.bn_aggr(ag[:, 2 * m:2 * m + 2], st[:])

                # shard (mean, var) -> (sum, sumsq)/B so the cross-core
                # reduction directly yields E[h] and E[h^2]
                mean_ap = ag[:].rearrange("p (m two) -> p m two", two=2)[:, :, 0]
                var_ap = ag[:].rearrange("p (m two) -> p m two", two=2)[:, :, 1]
                sums = sp.tile([128, 2 * MT], f32, tag="sums")
                nc.vector.tensor_scalar_mul(sums[:, 0:MT], mean_ap,
                                            float(C) / B)
                msq = sp.tile([128, MT], f32, tag="msq")
                nc.vector.tensor_mul(msq[:], mean_ap, mean_ap)
                nc.vector.tensor_add(sums[:, MT:2 * MT], var_ap, msq[:])
                nc.vector.tensor_scalar_mul(sums[:, MT:2 * MT],
                                            sums[:, MT:2 * MT], float(C) / B)

                agin = dp.tile([128, 2 * MT], f32, tag="arin")
                agout = dp.tile([n_cores * 128, 2 * MT], f32, tag="arout",
                                addr_space="Shared")
                nc.gpsimd.dma_start(agin[:], sums[:])
                nc.gpsimd.collective_compute(
                    "AllGather", ALU.bypass, replica_groups=rg,
                    ins=[agin.opt()], outs=[agout.opt()])
                # bring all shards back as [p, r, s] and reduce over ranks
                gall = sp.tile([128, n_cores * 2 * MT], f32, tag="gall")
                nc.gpsimd.dma_start(
                    gall[:].rearrange("p (r s) -> p r s", s=2 * MT),
                    agout[:].rearrange("(r p) s -> p r s", p=128))
                gst = sp.tile([128, 2 * MT], f32, tag="gst")
                nc.vector.tensor_reduce(
                    gst[:],
                    gall[:].rearrange("p (r s) -> p s r", s=2 * MT),
                    axis=AX.X, op=ALU.add)

                # a = gamma * rsqrt(var + eps); b = beta - mean * a
                gm = gst[:, 0:MT]
                ge2 = gst[:, MT:2 * MT]
                gmsq = sp.tile([128, MT], f32, tag="gmsq")
                nc.vector.tensor_mul(gmsq[:], gm, gm)
                gve = sp.tile([128, MT], f32, tag="gve")
                nc.vector.tensor_sub(gve[:], ge2, gmsq[:])
                nc.vector.tensor_scalar_add(gve[:], gve[:], BN_EPS)
                gstd = sp.tile([128, MT], f32, tag="gstd")
                nc.scalar.sqrt(gstd[:], gve[:])
                ginv = sp.tile([128, MT], f32, tag="ginv")
                nc.vector.reciprocal(ginv[:], gstd[:])
                aa = sp.tile([128, MT], f32, tag="aa")
                nc.vector.tensor_mul(aa[:], gT[:], ginv[:])
                mb = sp.tile([128, MT], f32, tag="mb")
                nc.vector.tensor_mul(mb[:], gm, aa[:])
                bb = sp.tile([128, MT], f32, tag="bb")
                nc.vector.tensor_sub(bb[:], bT[:], mb[:])

                # normalize chunk-wise so the next layer's matmuls start as
                # soon as the first chunk of every k-tile is ready
                nxt = [hp.tile([128, C], f32r, tag=f"hb{m}", name=f"hn_{l}_{m}")
                       for m in range(MT)]
                for n in range(NCH):
                    ncs = slice(n * NCHW, (n + 1) * NCHW)
                    for m in range(MT):
                        if m % 2 == 0:
                            nc.vector.tensor_scalar(
                                out=nxt[m][:, ncs], in0=outt[m][:, ncs],
                                scalar1=aa[:, m:m + 1], scalar2=bb[:, m:m + 1],
                                op0=ALU.mult, op1=ALU.add)
                        else:
                            # split normalize across DVE and ACT so the next
                            # layer's k-chain isn't paced by one engine
                            nc.scalar.activation(
                                nxt[m][:, ncs], outt[m][:, ncs], AF.Identity,
                                bias=bb[:, m:m + 1], scale=aa[:, m:m + 1])
                cur = nxt

            # ---- Final layer + output interleaved per chunk:
            #   mlp^T = Wout^T @ h + bout;
            #   out[:,0::2] = x1, out[:,1::2] = x2 + mlp ----
            wt = [wp.tile([128, L], f32r, tag=f"w{k}", name=f"wtout_{k}")
                  for k in range(MT)]
            for k in range(MT):
                nc.sync.dma_start(wt[k][:], wout_d[k * 128:(k + 1) * 128, :])
            boutT = sp.tile([128, LT], f32, tag="boutT")
            nc.sync.dma_start(boutT[:],
                              bout_d[:].rearrange("(m p) -> p m", p=128))
            yt = [hp.tile([128, C], f32, tag=f"ha{j}", name=f"yt_{j}")
                  for j in range(LT)]
            for n in range(NCH):
                ncs = slice(n * NCHW, (n + 1) * NCHW)
                for m in range(LT):
                    wm = 128 if m < LT - 1 else LREM
                    ps = pp.tile([128, NCHW], f32, tag="mm")
                    for k in range(MT):
                        nc.tensor.matmul(
                            ps[0:wm, :], wt[k][:, m * 128:m * 128 + wm],
                            cur[k][:, ncs],
                            start=(k == 0), stop=(k == MT - 1))
                    nc.scalar.activation(yt[m][0:wm, ncs], ps[0:wm, :],
                                         AF.Identity,
                                         bias=boutT[0:wm, m:m + 1], scale=1.0)
                for b in range(n * BPC, (n + 1) * BPC):
                    xin = xp.tile([128, D], f32, tag="xin2", bufs=4)
                    nc.sync.dma_start(xin[:], x_d[b * 128:(b + 1) * 128, :])
                    xo = xp.tile([128, D], f32, tag="xo", bufs=4)
                    xin_il = xin[:].rearrange("p (l two) -> p l two", two=2)
                    xo_il = xo[:].rearrange("p (l two) -> p l two", two=2)
                    nc.scalar.copy(xo_il[:, :, 0], xin_il[:, :, 0])
                    for j in range(LT):
                        wj = 128 if j < LT - 1 else LREM
                        ps = pp.tile([128, 128], f32, tag="tr")
                        nc.tensor.transpose(
                            ps[:, 0:wj], yt[j][0:wj, b * 128:(b + 1) * 128],
                            ident[0:wj, 0:wj])
                        nc.vector.tensor_add(
                            xo_il[:, j * 128:j * 128 + wj, 1],
                            ps[:, 0:wj],
                            xin_il[:, j * 128:j * 128 + wj, 1])
                    nc.sync.dma_start(out_d[b * 128:(b + 1) * 128, :], xo[:])

    nc.compile()
    return nc


def make_in_maps(x, Win, bin_, Wh, bh, gamma, beta, Wout, bout,
                 B=B_FULL, D=D_FULL, H=H_FULL, n_cores=NCORES):
    L = D // 2
    C = B // n_cores
    LT = (L + 127) // 128
    x = np.ascontiguousarray(np.asarray(x, dtype=np.float32))
    win_p = np.zeros((LT * 128, H), dtype=np.float32)
    win_p[:L] = np.asarray(Win, dtype=np.float32)
    bout_p = np.zeros((LT * 128,), dtype=np.float32)
    bout_p[:L] = np.asarray(bout, dtype=np.float32)
    common = {
        "win": win_p,
        "wh": np.ascontiguousarray(np.asarray(Wh, dtype=np.float32)),
        "wout": np.ascontiguousarray(np.asarray(Wout, dtype=np.float32)),
        "bin": np.asarray(bin_, dtype=np.float32),
        "bh": np.ascontiguousarray(np.asarray(bh, dtype=np.float32)),
        "gamma": np.ascontiguousarray(np.asarray(gamma, dtype=np.float32)),
        "beta": np.ascontiguousarray(np.asarray(beta, dtype=np.float32)),
        "bout": bout_p,
    }
    return [
        {"x": np.ascontiguousarray(x[c * C:(c + 1) * C]), **common}
        for c in range(n_cores)
    ]


_built = None


def kernel(x, Win, bin_, Wh, bh, gamma, beta, Wout, bout):
    global _built
    from concourse.bass_utils import run_bass_kernel_spmd

    if _built is None:
        _built = build_kernel()
    in_maps = make_in_maps(x, Win, bin_, Wh, bh, gamma, beta, Wout, bout)
    res = run_bass_kernel_spmd(_built, in_maps, core_ids=list(range(NCORES)))
    return np.concatenate([r["out"] for r in res.results], axis=0)

